# revision 22
# baseline (speedup 1.0000x reference)
"""Bass/Trainium2 kernel for nn_BellmanLoss (8-core data-parallel), v2.

Math: loss = sum_i (Q0[i, a_i] - target_i)^2 where a_i = first-argmax of
actions[i], target_i = r_i + 0.9 * max_a Qn[i,a] * (1 - done_i),
done_i = (states1[i,0] == 666).

Design (per core, 8192 rows):
- Feature-major MLP, CH=512 batch cols per tick, T=32 ticks (16 chunks x 2
  states interleaved).
- mm1: bf16, 2 MMs N=512 -> h1p [128,2,512] (2 PSUM banks, bufs=2).
- relu1: one [128,1024] op per tick (ACT/DVE balanced greedily).
- mm2: fp8 DoubleRow (2 MMs) or bf16 (4 MMs) -> h2p per-m-half [128,512]
  (1 bank, bufs=3). relu2: two [128,512] ops per tick.
- mm3: Q0 (pad 32) + Qn (pad 32) per pair; 2 pairs stacked into one qt
  [128,512] PSUM bank via tile_position col groups; fp8-DR or bf16.
- stack (ACT) -> PE transpose x4 -> qb copy (DVE) -> qbuf batch-major.
- Epilogue on GPSIMD (SBUF only) + DVE: first-argmax via score=32*a-iota,
  onehot select, maxqn, target, sum of squares -> [128,1] per core, host
  sums 1024 values.
- Host prep is layout-only (transpose/reshape/cast) + zeroing the 666
  sentinel in states1 feature 0 (done rows' Qn is multiplied by 0).
"""

import os
import numpy as np
import ml_dtypes

import concourse.bass as bass
import concourse.mybir as mybir
import concourse.tile as tile
from concourse import bacc
from concourse.bass_utils import run_bass_kernel_spmd

B, S, H, A = 65536, 128, 256, 18
NCORES = 8
BC = B // NCORES            # 8192 rows per core
CH = 512                    # batch cols per tick
NCH = BC // CH              # 16 chunks
T = 2 * NCH                 # 32 ticks (chunk-state interleave)
NQG = NCH // 2              # 8 qgroups (2 chunk-pairs each)
GI = NQG * 8                # 64 epilogue groups of 128 rows
LOADCOLS = 2048
NLOAD = BC // LOADCOLS      # 4 loads per state
DONE = 666.0
DISC = 0.9

F32 = mybir.dt.float32
BF16 = mybir.dt.bfloat16
FP8 = mybir.dt.float8e4
I32 = mybir.dt.int32
AF = mybir.ActivationFunctionType
OP = mybir.AluOpType
AX = mybir.AxisListType
DR = mybir.MatmulPerfMode.DoubleRow

NPBF = ml_dtypes.bfloat16
NPF8 = ml_dtypes.float8_e4m3

L1_FP8 = os.environ.get("BL_L1", "bf16") == "fp8"
L2_FP8 = os.environ.get("BL_L2", "fp8") == "fp8"
L3_FP8 = os.environ.get("BL_L3", "bf16") == "fp8"


def _build_program():
    nc = bacc.Bacc("TRN2", target_bir_lowering=False, debug=False)

    XDT = FP8 if L1_FP8 else BF16
    XP = 64 if L1_FP8 else 128
    XNI = 2 if L1_FP8 else 1
    x0t = nc.dram_tensor("x0t", [XP, XNI * BC], XDT,
                         kind="ExternalInput").ap()
    x1t = nc.dram_tensor("x1t", [XP, XNI * BC], XDT,
                         kind="ExternalInput").ap()
    actb = nc.dram_tensor("actb", [128, GI * A], F32, kind="ExternalInput").ap()
    rewb = nc.dram_tensor("rewb", [128, GI], F32, kind="ExternalInput").ap()
    s1b = nc.dram_tensor("s1b", [128, GI], F32, kind="ExternalInput").ap()
    if L1_FP8:
        w1d = nc.dram_tensor("w1d", [64, 2 * H], FP8, kind="ExternalInput").ap()
    else:
        w1d = nc.dram_tensor("w1d", [S, H], BF16, kind="ExternalInput").ap()
    # w2: DR layout [128,2,128] per m-half (fp8) or [256,256] bf16
    if L2_FP8:
        w2d = nc.dram_tensor("w2d", [128, 2 * H], FP8, kind="ExternalInput").ap()
    else:
        w2d = nc.dram_tensor("w2d", [H, H], BF16, kind="ExternalInput").ap()
    if L3_FP8:
        w3d = nc.dram_tensor("w3d", [128, 2 * 32], FP8, kind="ExternalInput").ap()
    else:
        w3d = nc.dram_tensor("w3d", [H, 32], BF16, kind="ExternalInput").ap()
    iotad = nc.dram_tensor("iotad", [128, A], F32, kind="ExternalInput").ap()
    identd = nc.dram_tensor("identd", [128, 128], BF16, kind="ExternalInput").ap()
    outp = nc.dram_tensor("outp", [128, GI], F32, kind="ExternalOutput").ap()

    H2DT = FP8 if L3_FP8 else BF16   # dtype of relu2 output (mm3 moving)
    H1DT = FP8 if L2_FP8 else BF16   # dtype of relu1 output (mm2 moving)

    from contextlib import ExitStack

    with tile.TileContext(nc) as tc, ExitStack() as ctx:
        singles = ctx.enter_context(tc.tile_pool(name="singles", bufs=1))
        xpool = ctx.enter_context(tc.tile_pool(name="xpool", bufs=2))
        h1spool = ctx.enter_context(tc.tile_pool(name="h1spool", bufs=4))
        h2spool = ctx.enter_context(tc.tile_pool(name="h2spool", bufs=6))
        qtspool = ctx.enter_context(tc.tile_pool(name="qtspool", bufs=3))
        big = ctx.enter_context(tc.tile_pool(name="big", bufs=1))
        ps_h1 = ctx.enter_context(tc.tile_pool(name="ps_h1", bufs=2, space="PSUM"))
        ps_h2 = ctx.enter_context(tc.tile_pool(name="ps_h2", bufs=3, space="PSUM"))
        ps_q = ctx.enter_context(tc.tile_pool(name="ps_q", bufs=1, space="PSUM"))

        # --- weights / constants ---
        if L1_FP8:
            w1_s = []
            for m in range(2):
                t_ = singles.tile([64, 2, 128], FP8, tag=f"w1_{m}")
                nc.gpsimd.dma_start(
                    out=t_[:, :, :].rearrange("p a b -> p (a b)"),
                    in_=w1d[:, m * 256:(m + 1) * 256])
                w1_s.append(t_)
        else:
            w1t = singles.tile([S, H], BF16, tag="w1")
            nc.gpsimd.dma_start(out=w1t[:, 0:128], in_=w1d[:, 0:128])
            nc.scalar.dma_start(out=w1t[:, 128:256], in_=w1d[:, 128:256])
            w1_s = [w1t[:, 0:128], w1t[:, 128:256]]
        w2_s = []
        if L2_FP8:
            for m in range(2):
                t_ = singles.tile([128, 2, 128], FP8, tag=f"w2_{m}")
                nc.scalar.dma_start(out=t_[:, :, :].rearrange("p a b -> p (a b)"),
                                    in_=w2d[:, m * 256:(m + 1) * 256])
                w2_s.append(t_)
        else:
            for k in range(2):
                t_ = singles.tile([128, H], BF16, tag=f"w2_{k}")
                nc.scalar.dma_start(out=t_, in_=w2d[k * 128:(k + 1) * 128, :])
                w2_s.append(t_)
        if L3_FP8:
            w3_s = singles.tile([128, 2, 32], FP8, tag="w3")
            nc.scalar.dma_start(out=w3_s[:, :, :].rearrange("p a b -> p (a b)"), in_=w3d)
        else:
            w3_s = []
            for k in range(2):
                t_ = singles.tile([128, 32], BF16, tag=f"w3_{k}")
                nc.scalar.dma_start(out=t_, in_=w3d[k * 128:(k + 1) * 128, :])
                w3_s.append(t_)
        iota_s = singles.tile([128, A], F32, tag="iota")
        nc.gpsimd.dma_start(out=iota_s, in_=iotad)
        ident = singles.tile([128, 128], BF16, tag="ident")
        nc.gpsimd.dma_start(out=ident, in_=identd)
        actb_s = singles.tile([128, GI * A], F32, tag="actb")
        rewb_s = singles.tile([128, GI], F32, tag="rewb")
        s1b_s = singles.tile([128, GI], F32, tag="s1b")

        qbuf = big.tile([128, NQG * 512], BF16, tag="qbuf")

        # --- static ACT/DVE assignment ---
        def relu_op(dst, src, eng):
            if eng == "act":
                nc.scalar.activation(dst, src, AF.Relu, scale=1.0)
            else:
                nc.vector.tensor_scalar(dst, src, 0.0, None, OP.max)

        def copy_op(dst, src, eng):
            if eng == "act":
                nc.scalar.activation(dst, src, AF.Copy, scale=1.0)
            else:
                nc.vector.tensor_copy(dst, src)

        # --- pipeline state ---
        xL_tiles = {}
        h1p_t, h1s_t, h2p_t, h2s_t = {}, {}, {}, {}
        qt_g, qts_g, tp_g = {}, {}, {}

        NI = XNI
        x0v = x0t[:, :].rearrange("p (i n) -> p i n", i=NI)
        x1v = x1t[:, :].rearrange("p (i n) -> p i n", i=NI)

        def do_dma(li, split=False):
            x0L = xpool.tile([XP, NI, LOADCOLS], XDT, tag="x0",
                             name=f"x0L_{li}")
            x1L = xpool.tile([XP, NI, LOADCOLS], XDT, tag="x1",
                             name=f"x1L_{li}")
            o = li * LOADCOLS
            if split:
                # progressive pieces so early ticks start ASAP
                for (a, b) in ((0, 512), (512, 1024), (1024, 2048)):
                    nc.sync.dma_start(out=x0L[:, :, a:b],
                                      in_=x0v[:, :, o + a:o + b])
                    nc.sync.dma_start(out=x1L[:, :, a:b],
                                      in_=x1v[:, :, o + a:o + b])
            else:
                nc.sync.dma_start(out=x0L, in_=x0v[:, :, o:o + LOADCOLS])
                nc.sync.dma_start(out=x1L, in_=x1v[:, :, o:o + LOADCOLS])
            xL_tiles[li] = (x0L, x1L)

        def xs_for(t):
            c, s = t // 2, t % 2
            li = (c * CH) // LOADCOLS
            ci = (c * CH) % LOADCOLS // CH
            xl = xL_tiles[li][s]
            if L1_FP8:
                return xl[:, :, ci * CH:(ci + 1) * CH]
            return xl[:, 0, ci * CH:(ci + 1) * CH]

        def st_mm1(t):
            h1p = ps_h1.tile([128, 2, CH], F32, tag="h1p", name=f"h1p_{t}")
            xs = xs_for(t)
            for m in range(2):
                nc.tensor.matmul(h1p[:, m, :], w1_s[m], xs, start=True,
                                 stop=True,
                                 perf_mode=DR if L1_FP8 else None)
            h1p_t[t] = h1p

        def st_relu1(t):
            h1s = h1spool.tile([128, 2, CH], H1DT, tag="h1s", name=f"h1s_{t}")
            relu_op(h1s[:, :, :].rearrange("p a b -> p (a b)"),
                    h1p_t.pop(t)[:, :, :].rearrange("p a b -> p (a b)"),
                    "dve")
            h1s_t[t] = h1s

        def st_mm2(t):
            h1s = h1s_t.pop(t)
            tiles = []
            for m in range(2):
                h2p = ps_h2.tile([128, CH], F32, tag="h2p", name=f"h2p_{t}_{m}")
                if L2_FP8:
                    nc.tensor.matmul(h2p, w2_s[m], h1s, start=True, stop=True,
                                     perf_mode=DR)
                else:
                    for k in range(2):
                        nc.tensor.matmul(h2p, w2_s[k][:, m * 128:(m + 1) * 128],
                                         h1s[:, k, :], start=(k == 0),
                                         stop=(k == 1))
                tiles.append(h2p)
            h2p_t[t] = tiles

        def st_relu2(t):
            tiles = h2p_t.pop(t)
            h2s = h2spool.tile([128, 2, CH], H2DT, tag="h2s", name=f"h2s_{t}")
            for m in range(2):
                relu_op(h2s[:, m, :], tiles[m], "act")
            h2s_t[t] = h2s

        def st_mm3(c):
            # pair c: Q0 from state0 tick 2c, Qn from state1 tick 2c+1
            g, pp = c // 2, c % 2
            if pp == 0:
                qt_g[g] = ps_q.tile([128, CH], F32, tag="q", name=f"qt_{g}")
            qt = qt_g[g]
            h2s0 = h2s_t.pop(2 * c)
            h2s1 = h2s_t.pop(2 * c + 1)
            po = pp * 64
            if L3_FP8:
                nc.tensor.matmul(qt[po:po + 32, :], w3_s, h2s0, start=True,
                                 stop=True, perf_mode=DR, tile_position=(0, po))
                nc.tensor.matmul(qt[po + 32:po + 64, :], w3_s, h2s1, start=True,
                                 stop=True, perf_mode=DR,
                                 tile_position=(0, po + 32))
            else:
                for k in range(2):
                    nc.tensor.matmul(qt[po:po + 32, :], w3_s[k], h2s0[:, k, :],
                                     start=(k == 0), stop=(k == 1),
                                     tile_position=(0, po))
                for k in range(2):
                    nc.tensor.matmul(qt[po + 32:po + 64, :], w3_s[k],
                                     h2s1[:, k, :], start=(k == 0),
                                     stop=(k == 1), tile_position=(0, po + 32))

        def st_stack(g):
            qts = qtspool.tile([128, CH], BF16, tag="qts", name=f"qts_{g}")
            copy_op(qts, qt_g.pop(g), "act")
            qts_g[g] = qts

        def st_tp(g):
            tp = ps_q.tile([128, 4, 128], BF16, tag="q", name=f"tp_{g}")
            qts = qts_g.pop(g)
            for j in range(4):
                nc.tensor.transpose(tp[:, j, :], qts[:, j * 128:(j + 1) * 128],
                                    ident)
            tp_g[g] = tp

        def st_qb(g):
            copy_op(qbuf[:, g * 512:(g + 1) * 512],
                    tp_g.pop(g)[:, :, :].rearrange("p a b -> p (a b)"), "dve")

        # --- epilogue tiles ---
        score = big.tile([128, GI * A], F32, tag="score")
        rowmax = big.tile([128, GI], F32, tag="rowmax")
        ss = big.tile([128, GI * A], F32, tag="ss")
        cmb = big.tile([128, GI * A], F32, tag="cmb")
        q0sel = big.tile([128, GI], F32, tag="q0sel")
        maxqn = big.tile([128, GI], F32, tag="maxqn")
        donem = big.tile([128, GI], F32, tag="donem")
        fac = big.tile([128, GI], F32, tag="fac")
        t1 = big.tile([128, GI], F32, tag="t1")
        t2 = big.tile([128, GI], F32, tag="t2")
        diff = big.tile([128, GI], F32, tag="diff")
        sq = big.tile([128, GI], F32, tag="sq")

        a3 = lambda t_: t_[:, :].rearrange("p (g a) -> p g a", a=A)
        qb3 = qbuf[:, :].rearrange("p (gi c) -> p gi c", c=64)
        NQ = 4
        HG = GI // NQ

        def ep_front(hh):
            # score = 512*a - 16*idx (host provides actb*512, iota=16*idx);
            # q0sel later = max_a(Q0[a] + score[a] - rowmax): exact select of
            # the first-argmax action's Q0 (gaps >= 16 dominate |Q| <= ~8).
            gsl = slice(hh * HG, (hh + 1) * HG)
            iot_b = iota_s[:, None, :].broadcast_to([128, HG, A])
            nc.gpsimd.tensor_tensor(a3(score)[:, gsl], a3(actb_s)[:, gsl],
                                    iot_b, OP.subtract)
            nc.vector.tensor_reduce(rowmax[:, gsl], a3(score)[:, gsl], AX.X,
                                    OP.max)
            nc.gpsimd.tensor_tensor(
                a3(ss)[:, gsl], a3(score)[:, gsl],
                rowmax[:, gsl, None].broadcast_to([128, HG, A]), OP.subtract)

        def ep_tail(hh):
            gsl = slice(hh * HG, (hh + 1) * HG)
            last = hh == NQ - 1
            tt1 = nc.vector.tensor_tensor if last else nc.gpsimd.tensor_tensor
            if last:
                nc.vector.tensor_tensor(a3(cmb)[:, gsl], a3(ss)[:, gsl],
                                        qb3[:, gsl, 0:A], OP.add)
            else:
                nc.gpsimd.tensor_tensor(a3(cmb)[:, gsl], a3(ss)[:, gsl],
                                        qb3[:, gsl, 0:A], OP.add)
            nc.vector.tensor_reduce(q0sel[:, gsl], a3(cmb)[:, gsl], AX.X,
                                    OP.max)
            nc.vector.tensor_reduce(maxqn[:, gsl], qb3[:, gsl, 32:32 + A],
                                    AX.X, OP.max)
            tt1(t1[:, gsl], maxqn[:, gsl], fac[:, gsl], OP.mult)
            tt1(t2[:, gsl], t1[:, gsl], rewb_s[:, gsl], OP.add)
            tt1(diff[:, gsl], q0sel[:, gsl], t2[:, gsl], OP.subtract)
            tt1(sq[:, gsl], diff[:, gsl], diff[:, gsl], OP.mult)

        # --- main software-pipelined loop ---
        do_dma(0, split=True)
        PASS_PER_LOAD = 2 * LOADCOLS // CH   # 8 ticks per load
        tails_done = 0
        for t in range(T + 12):
            nt = t + 6
            if nt < T and nt % PASS_PER_LOAD == 0:
                do_dma(nt // PASS_PER_LOAD)
            if t == 0:
                nc.gpsimd.dma_start(out=actb_s, in_=actb)
                nc.gpsimd.dma_start(out=rewb_s, in_=rewb)
                nc.gpsimd.dma_start(out=s1b_s, in_=s1b)
            if t >= 6 and t % 2 == 0 and (t - 6) // 2 < NQ:
                ep_front((t - 6) // 2)
            if t == 10:
                nc.vector.tensor_scalar(donem, s1b_s, DONE, None, OP.is_equal)
                nc.vector.tensor_scalar(fac, donem, -DISC, DISC, OP.mult,
                                        OP.add)
            if 0 <= t - 3 < T:
                st_relu2(t - 3)
            if 0 <= t - 1 < T:
                st_relu1(t - 1)
            if t < T:
                st_mm1(t)
            if 0 <= t - 2 < T:
                st_mm2(t - 2)
            tg = t - 10
            if tg >= 0 and tg % 4 == 0 and tg // 4 < NQG:
                st_tp(tg // 4)
            if 0 <= t - 5 < T and (t - 5) % 2 == 1:
                st_mm3((t - 5) // 2)
            tg = t - 9
            if tg >= 0 and tg % 4 == 0 and tg // 4 < NQG:
                st_stack(tg // 4)
            tg = t - 11
            if tg >= 0 and tg % 4 == 0 and tg // 4 < NQG:
                g = tg // 4
                st_qb(g)
                while (tails_done < NQ - 1
                       and (g + 1) * 8 >= (tails_done + 1) * HG):
                    ep_tail(tails_done)
                    tails_done += 1
                    if tails_done == NQ - 1:
                        nc.sync.dma_start(out=outp[:, 0:3 * HG],
                                          in_=sq[:, 0:3 * HG])
        while tails_done < NQ:
            ep_tail(tails_done)
            tails_done += 1
            if tails_done == NQ - 1:
                nc.sync.dma_start(out=outp[:, 0:3 * HG],
                                  in_=sq[:, 0:3 * HG])
        nc.sync.dma_start(out=outp[:, 3 * HG:GI], in_=sq[:, 3 * HG:GI])

    nc.compile()
    return nc


_CACHE = {}


def _get_program():
    if "nc" not in _CACHE:
        _CACHE["nc"] = _build_program()
    return _CACHE["nc"]


def _prep_in_maps(inputs):
    st0 = np.asarray(inputs["states0"], dtype=np.float32)
    st1 = np.asarray(inputs["states1"], dtype=np.float32)
    act = np.asarray(inputs["actions"], dtype=np.int32)
    rew = np.asarray(inputs["rewards"], dtype=np.float32)
    W1 = np.asarray(inputs["W1"], dtype=np.float32)
    W2 = np.asarray(inputs["W2"], dtype=np.float32)
    W3 = np.asarray(inputs["W3"], dtype=np.float32)

    s1_feat0 = st1[:, 0].copy()
    st1m = st1.copy()
    st1m[:, 0] = np.where(s1_feat0 == DONE, 0.0, s1_feat0)

    # mm1-DR layouts: x_dr[p, i, n] = x[n, i*64+p]; w1dr[p, m, i, mo] =
    # W1[i*64+p, m*128+mo]
    if L1_FP8:
        def xdr(st):
            xt = st.T.reshape(2, 64, B)  # [i, p, row]
            out = np.ascontiguousarray(xt.transpose(1, 0, 2))  # [64, 2, B]
            return np.clip(out, -240.0, 240.0).astype(NPF8)
        x0a = xdr(st0)
        x1a = xdr(st1m)
        x0dr = [np.ascontiguousarray(
                    x0a[:, :, c * BC:(c + 1) * BC]).reshape(64, 2 * BC)
                for c in range(NCORES)]
        x1dr = [np.ascontiguousarray(
                    x1a[:, :, c * BC:(c + 1) * BC]).reshape(64, 2 * BC)
                for c in range(NCORES)]
        w1p = np.empty((64, 2, 2, 128), np.float32)
        for m in range(2):
            for i in range(2):
                w1p[:, m, i, :] = W1[i * 64:i * 64 + 64,
                                     m * 128:(m + 1) * 128]
        w1b = w1p.reshape(64, 512).astype(NPF8)
    else:
        x0a = st0.T.astype(NPBF)
        x1a = st1m.T.astype(NPBF)
        x0dr = [np.ascontiguousarray(x0a[:, c * BC:(c + 1) * BC])
                for c in range(NCORES)]
        x1dr = [np.ascontiguousarray(x1a[:, c * BC:(c + 1) * BC])
                for c in range(NCORES)]
        w1b = W1.astype(NPBF)
    if L2_FP8:
        # [128, 2, 2*128]: w2d[:, m*256+(i*128+mo)] = W2[i*128+p, m*128+mo]
        w2p = np.empty((128, 2, 2, 128), np.float32)
        for m in range(2):
            for i in range(2):
                w2p[:, m, i, :] = W2[i * 128:(i + 1) * 128,
                                     m * 128:(m + 1) * 128]
        w2prep = w2p.reshape(128, 512).astype(NPF8)
    else:
        w2prep = W2.astype(NPBF)
    if L3_FP8:
        w3p = np.zeros((128, 2, 32), np.float32)
        for i in range(2):
            w3p[:, i, 0:A] = W3[i * 128:(i + 1) * 128, :]
        w3prep = w3p.reshape(128, 64).astype(NPF8)
    else:
        w3p = np.zeros((H, 32), np.float32)
        w3p[:, 0:A] = W3
        w3prep = w3p.astype(NPBF)

    iota = np.ascontiguousarray(
        np.broadcast_to(np.arange(A, dtype=np.float32) * 16.0, (128, A)))
    ident = np.eye(128, dtype=np.float32).astype(NPBF)

    # epilogue row permutation: row(r', gi) with gi=(g,j,p):
    #   g=gi//8, j=(gi%8)//2, p=gi%2 -> row = 512*(2g+p) + 128j + r'
    gi_idx = np.arange(GI)
    g_, j_, p_ = gi_idx // 8, (gi_idx % 8) // 2, gi_idx % 2
    base = 512 * (2 * g_ + p_) + 128 * j_          # [GI]
    rows = base[None, :] + np.arange(128)[:, None]  # [128, GI]

    in_maps = []
    for c in range(NCORES):
        r0, r1 = c * BC, (c + 1) * BC
        act_c = act[r0:r1]
        rew_c = rew[r0:r1]
        s1f_c = s1_feat0[r0:r1]
        in_maps.append({
            "x0t": x0dr[c], "x1t": x1dr[c],
            "actb": np.ascontiguousarray(
                act_c[rows].astype(np.float32).reshape(128, GI * A) * 512.0),
            "rewb": np.ascontiguousarray(rew_c[rows]),
            "s1b": np.ascontiguousarray(s1f_c[rows]),
            "w1d": w1b, "w2d": w2prep, "w3d": w3prep,
            "iotad": iota, "identd": ident,
        })
    return in_maps


def _run(inputs, trace=False):
    nc = _get_program()
    in_maps = _prep_in_maps(inputs)
    res = run_bass_kernel_spmd(nc, in_maps, core_ids=list(range(NCORES)),
                               trace=trace)
    total = 0.0
    for r in res.results:
        total += float(np.asarray(r["outp"], dtype=np.float64).sum())
    return np.array(np.float32(total)), res


def kernel(**inputs) -> np.ndarray:
    val, _ = _run(inputs, trace=False)
    return val


# revision 23
# speedup vs baseline: 1.1674x; 1.1674x over previous
"""Bass/Trainium2 kernel for nn_BellmanLoss (8-core data-parallel), v2.

Math: loss = sum_i (Q0[i, a_i] - target_i)^2 where a_i = first-argmax of
actions[i], target_i = r_i + 0.9 * max_a Qn[i,a] * (1 - done_i),
done_i = (states1[i,0] == 666).

Design (per core, 8192 rows):
- Feature-major MLP, CH=512 batch cols per tick, T=32 ticks (16 chunks x 2
  states interleaved).
- mm1: bf16, 2 MMs N=512 -> h1p [128,2,512] (2 PSUM banks, bufs=2).
- relu1: one [128,1024] op per tick (ACT/DVE balanced greedily).
- mm2: fp8 DoubleRow (2 MMs) or bf16 (4 MMs) -> h2p per-m-half [128,512]
  (1 bank, bufs=3). relu2: two [128,512] ops per tick.
- mm3: Q0 (pad 32) + Qn (pad 32) per pair; 2 pairs stacked into one qt
  [128,512] PSUM bank via tile_position col groups; fp8-DR or bf16.
- stack (ACT) -> PE transpose x4 -> qb copy (DVE) -> qbuf batch-major.
- Epilogue on GPSIMD (SBUF only) + DVE: first-argmax via score=32*a-iota,
  onehot select, maxqn, target, sum of squares -> [128,1] per core, host
  sums 1024 values.
- Host prep is layout-only (transpose/reshape/cast) + zeroing the 666
  sentinel in states1 feature 0 (done rows' Qn is multiplied by 0).
"""

import os
import numpy as np
import ml_dtypes

import concourse.bass as bass
import concourse.mybir as mybir
import concourse.tile as tile
from concourse import bacc
from concourse.bass_utils import run_bass_kernel_spmd

B, S, H, A = 65536, 128, 256, 18
NCORES = 8
BC = B // NCORES            # 8192 rows per core
CH = 512                    # batch cols per tick
NCH = BC // CH              # 16 chunks
T = 2 * NCH                 # 32 ticks (chunk-state interleave)
NQG = NCH // 2              # 8 qgroups (2 chunk-pairs each)
GI = NQG * 8                # 64 epilogue groups of 128 rows
LOADCOLS = 2048
NLOAD = BC // LOADCOLS      # 4 loads per state
DONE = 666.0
DISC = 0.9

F32 = mybir.dt.float32
BF16 = mybir.dt.bfloat16
FP8 = mybir.dt.float8e4
I32 = mybir.dt.int32
AF = mybir.ActivationFunctionType
OP = mybir.AluOpType
AX = mybir.AxisListType
DR = mybir.MatmulPerfMode.DoubleRow

NPBF = ml_dtypes.bfloat16
NPF8 = ml_dtypes.float8_e4m3

L1_FP8 = os.environ.get("BL_L1", "bf16") == "fp8"
L2_FP8 = os.environ.get("BL_L2", "fp8") == "fp8"
L3_FP8 = os.environ.get("BL_L3", "bf16") == "fp8"


def _build_program():
    nc = bacc.Bacc("TRN2", target_bir_lowering=False, debug=False)

    XDT = FP8 if L1_FP8 else BF16
    XP = 64 if L1_FP8 else 128
    XNI = 2 if L1_FP8 else 1
    x0t = nc.dram_tensor("x0t", [XP, XNI * BC], XDT,
                         kind="ExternalInput").ap()
    x1t = nc.dram_tensor("x1t", [XP, XNI * BC], XDT,
                         kind="ExternalInput").ap()
    actb = nc.dram_tensor("actb", [128, GI * A], F32, kind="ExternalInput").ap()
    rewb = nc.dram_tensor("rewb", [128, GI], F32, kind="ExternalInput").ap()
    s1b = nc.dram_tensor("s1b", [128, GI], F32, kind="ExternalInput").ap()
    if L1_FP8:
        w1d = nc.dram_tensor("w1d", [64, 2 * H], FP8, kind="ExternalInput").ap()
    else:
        w1d = nc.dram_tensor("w1d", [S, H], BF16, kind="ExternalInput").ap()
    # w2: DR layout [128,2,128] per m-half (fp8) or [256,256] bf16
    if L2_FP8:
        w2d = nc.dram_tensor("w2d", [128, 2 * H], FP8, kind="ExternalInput").ap()
    else:
        w2d = nc.dram_tensor("w2d", [H, H], BF16, kind="ExternalInput").ap()
    if L3_FP8:
        w3d = nc.dram_tensor("w3d", [128, 2 * 32], FP8, kind="ExternalInput").ap()
    else:
        w3d = nc.dram_tensor("w3d", [H, 32], BF16, kind="ExternalInput").ap()
    iotad = nc.dram_tensor("iotad", [128, A], F32, kind="ExternalInput").ap()
    identd = nc.dram_tensor("identd", [128, 128], BF16, kind="ExternalInput").ap()
    outp = nc.dram_tensor("outp", [128, GI], F32, kind="ExternalOutput").ap()

    H2DT = FP8 if L3_FP8 else BF16   # dtype of relu2 output (mm3 moving)
    H1DT = FP8 if L2_FP8 else BF16   # dtype of relu1 output (mm2 moving)

    from contextlib import ExitStack

    with tile.TileContext(nc) as tc, ExitStack() as ctx:
        singles = ctx.enter_context(tc.tile_pool(name="singles", bufs=1))
        xpool = ctx.enter_context(tc.tile_pool(name="xpool", bufs=2))
        h1spool = ctx.enter_context(tc.tile_pool(name="h1spool", bufs=4))
        h2spool = ctx.enter_context(tc.tile_pool(name="h2spool", bufs=6))
        qtspool = ctx.enter_context(tc.tile_pool(name="qtspool", bufs=3))
        big = ctx.enter_context(tc.tile_pool(name="big", bufs=1))
        ps_h1 = ctx.enter_context(tc.tile_pool(name="ps_h1", bufs=2, space="PSUM"))
        ps_h2 = ctx.enter_context(tc.tile_pool(name="ps_h2", bufs=3, space="PSUM"))
        ps_q = ctx.enter_context(tc.tile_pool(name="ps_q", bufs=1, space="PSUM"))

        # --- weights / constants ---
        if L1_FP8:
            w1_s = []
            for m in range(2):
                t_ = singles.tile([64, 2, 128], FP8, tag=f"w1_{m}")
                nc.gpsimd.dma_start(
                    out=t_[:, :, :].rearrange("p a b -> p (a b)"),
                    in_=w1d[:, m * 256:(m + 1) * 256])
                w1_s.append(t_)
        else:
            w1t = singles.tile([S, H], BF16, tag="w1")
            nc.gpsimd.dma_start(out=w1t[:, 0:128], in_=w1d[:, 0:128])
            nc.scalar.dma_start(out=w1t[:, 128:256], in_=w1d[:, 128:256])
            w1_s = [w1t[:, 0:128], w1t[:, 128:256]]
        w2_s = []
        if L2_FP8:
            for m in range(2):
                t_ = singles.tile([128, 2, 128], FP8, tag=f"w2_{m}")
                nc.scalar.dma_start(out=t_[:, :, :].rearrange("p a b -> p (a b)"),
                                    in_=w2d[:, m * 256:(m + 1) * 256])
                w2_s.append(t_)
        else:
            for k in range(2):
                t_ = singles.tile([128, H], BF16, tag=f"w2_{k}")
                nc.scalar.dma_start(out=t_, in_=w2d[k * 128:(k + 1) * 128, :])
                w2_s.append(t_)
        if L3_FP8:
            w3_s = singles.tile([128, 2, 32], FP8, tag="w3")
            nc.scalar.dma_start(out=w3_s[:, :, :].rearrange("p a b -> p (a b)"), in_=w3d)
        else:
            w3_s = []
            for k in range(2):
                t_ = singles.tile([128, 32], BF16, tag=f"w3_{k}")
                nc.scalar.dma_start(out=t_, in_=w3d[k * 128:(k + 1) * 128, :])
                w3_s.append(t_)
        iota_s = singles.tile([128, A], F32, tag="iota")
        nc.gpsimd.dma_start(out=iota_s, in_=iotad)
        ident = singles.tile([128, 128], BF16, tag="ident")
        nc.gpsimd.dma_start(out=ident, in_=identd)
        actb_s = singles.tile([128, GI * A], F32, tag="actb")
        rewb_s = singles.tile([128, GI], F32, tag="rewb")
        s1b_s = singles.tile([128, GI], F32, tag="s1b")

        qbuf = big.tile([128, NQG * 512], BF16, tag="qbuf")

        # --- static ACT/DVE assignment ---
        def relu_op(dst, src, eng):
            if eng == "act":
                nc.scalar.activation(dst, src, AF.Relu, scale=1.0)
            else:
                nc.vector.tensor_scalar(dst, src, 0.0, None, OP.max)

        def copy_op(dst, src, eng):
            if eng == "act":
                nc.scalar.activation(dst, src, AF.Copy, scale=1.0)
            else:
                nc.vector.tensor_copy(dst, src)

        # --- pipeline state ---
        xL_tiles = {}
        h1p_t, h1s_t, h2p_t, h2s_t = {}, {}, {}, {}
        qt_g, qts_g, tp_g = {}, {}, {}

        NI = XNI
        x0v = x0t[:, :].rearrange("p (i n) -> p i n", i=NI)
        x1v = x1t[:, :].rearrange("p (i n) -> p i n", i=NI)

        def do_dma(li, split=False):
            x0L = xpool.tile([XP, NI, LOADCOLS], XDT, tag="x0",
                             name=f"x0L_{li}")
            x1L = xpool.tile([XP, NI, LOADCOLS], XDT, tag="x1",
                             name=f"x1L_{li}")
            o = li * LOADCOLS
            if split:
                # progressive pieces so early ticks start ASAP
                for (a, b) in ((0, 512), (512, 1024), (1024, 2048)):
                    nc.sync.dma_start(out=x0L[:, :, a:b],
                                      in_=x0v[:, :, o + a:o + b])
                    nc.sync.dma_start(out=x1L[:, :, a:b],
                                      in_=x1v[:, :, o + a:o + b])
            else:
                nc.sync.dma_start(out=x0L, in_=x0v[:, :, o:o + LOADCOLS])
                nc.sync.dma_start(out=x1L, in_=x1v[:, :, o:o + LOADCOLS])
            xL_tiles[li] = (x0L, x1L)

        def xs_for(t):
            c, s = t // 2, t % 2
            li = (c * CH) // LOADCOLS
            ci = (c * CH) % LOADCOLS // CH
            xl = xL_tiles[li][s]
            if L1_FP8:
                return xl[:, :, ci * CH:(ci + 1) * CH]
            return xl[:, 0, ci * CH:(ci + 1) * CH]

        def st_mm1(t):
            h1p = ps_h1.tile([128, 2, CH], F32, tag="h1p", name=f"h1p_{t}")
            xs = xs_for(t)
            for m in range(2):
                nc.tensor.matmul(h1p[:, m, :], w1_s[m], xs, start=True,
                                 stop=True,
                                 perf_mode=DR if L1_FP8 else None)
            h1p_t[t] = h1p

        def st_relu1(t):
            h1s = h1spool.tile([128, 2, CH], H1DT, tag="h1s", name=f"h1s_{t}")
            relu_op(h1s[:, :, :].rearrange("p a b -> p (a b)"),
                    h1p_t.pop(t)[:, :, :].rearrange("p a b -> p (a b)"),
                    "dve")
            h1s_t[t] = h1s

        def st_mm2(t):
            h1s = h1s_t.pop(t)
            tiles = []
            for m in range(2):
                h2p = ps_h2.tile([128, CH], F32, tag="h2p", name=f"h2p_{t}_{m}")
                if L2_FP8:
                    nc.tensor.matmul(h2p, w2_s[m], h1s, start=True, stop=True,
                                     perf_mode=DR)
                else:
                    for k in range(2):
                        nc.tensor.matmul(h2p, w2_s[k][:, m * 128:(m + 1) * 128],
                                         h1s[:, k, :], start=(k == 0),
                                         stop=(k == 1))
                tiles.append(h2p)
            h2p_t[t] = tiles

        def st_relu2(t):
            tiles = h2p_t.pop(t)
            h2s = h2spool.tile([128, 2, CH], H2DT, tag="h2s", name=f"h2s_{t}")
            for m in range(2):
                relu_op(h2s[:, m, :], tiles[m], "act")
            h2s_t[t] = h2s

        def st_mm3(c):
            # pair c: Q0 from state0 tick 2c, Qn from state1 tick 2c+1
            g, pp = c // 2, c % 2
            if pp == 0:
                qt_g[g] = ps_q.tile([128, CH], F32, tag="q", name=f"qt_{g}")
            qt = qt_g[g]
            h2s0 = h2s_t.pop(2 * c)
            h2s1 = h2s_t.pop(2 * c + 1)
            po = pp * 64
            if L3_FP8:
                nc.tensor.matmul(qt[po:po + 32, :], w3_s, h2s0, start=True,
                                 stop=True, perf_mode=DR, tile_position=(0, po))
                nc.tensor.matmul(qt[po + 32:po + 64, :], w3_s, h2s1, start=True,
                                 stop=True, perf_mode=DR,
                                 tile_position=(0, po + 32))
            else:
                for k in range(2):
                    nc.tensor.matmul(qt[po:po + 32, :], w3_s[k], h2s0[:, k, :],
                                     start=(k == 0), stop=(k == 1),
                                     tile_position=(0, po))
                for k in range(2):
                    nc.tensor.matmul(qt[po + 32:po + 64, :], w3_s[k],
                                     h2s1[:, k, :], start=(k == 0),
                                     stop=(k == 1), tile_position=(0, po + 32))

        def st_stack(g):
            qts = qtspool.tile([128, CH], BF16, tag="qts", name=f"qts_{g}")
            copy_op(qts, qt_g.pop(g), "act")
            qts_g[g] = qts

        def st_tp(g):
            tp = ps_q.tile([128, 4, 128], BF16, tag="q", name=f"tp_{g}")
            qts = qts_g.pop(g)
            for j in range(4):
                nc.tensor.transpose(tp[:, j, :], qts[:, j * 128:(j + 1) * 128],
                                    ident)
            tp_g[g] = tp

        def st_qb(g):
            copy_op(qbuf[:, g * 512:(g + 1) * 512],
                    tp_g.pop(g)[:, :, :].rearrange("p a b -> p (a b)"), "dve")

        # --- epilogue tiles ---
        score = big.tile([128, GI * A], F32, tag="score")
        rowmax = big.tile([128, GI], F32, tag="rowmax")
        ss = big.tile([128, GI * A], F32, tag="ss")
        cmb = big.tile([128, GI * A], F32, tag="cmb")
        q0sel = big.tile([128, GI], F32, tag="q0sel")
        maxqn = big.tile([128, GI], F32, tag="maxqn")
        donem = big.tile([128, GI], F32, tag="donem")
        fac = big.tile([128, GI], F32, tag="fac")
        t1 = big.tile([128, GI], F32, tag="t1")
        t2 = big.tile([128, GI], F32, tag="t2")
        diff = big.tile([128, GI], F32, tag="diff")
        sq = big.tile([128, GI], F32, tag="sq")

        a3 = lambda t_: t_[:, :].rearrange("p (g a) -> p g a", a=A)
        qb3 = qbuf[:, :].rearrange("p (gi c) -> p gi c", c=64)
        NQ = 4
        HG = GI // NQ

        def ep_front(hh):
            # score = 512*a - 16*idx (host provides actb*512, iota=16*idx);
            # q0sel later = max_a(Q0[a] + score[a] - rowmax): exact select of
            # the first-argmax action's Q0 (gaps >= 16 dominate |Q| <= ~8).
            gsl = slice(hh * HG, (hh + 1) * HG)
            iot_b = iota_s[:, None, :].broadcast_to([128, HG, A])
            nc.gpsimd.tensor_tensor(a3(score)[:, gsl], a3(actb_s)[:, gsl],
                                    iot_b, OP.subtract)
            nc.vector.tensor_reduce(rowmax[:, gsl], a3(score)[:, gsl], AX.X,
                                    OP.max)
            nc.gpsimd.tensor_tensor(
                a3(ss)[:, gsl], a3(score)[:, gsl],
                rowmax[:, gsl, None].broadcast_to([128, HG, A]), OP.subtract)

        def ep_tail(hh):
            gsl = slice(hh * HG, (hh + 1) * HG)
            last = hh == NQ - 1
            tt1 = nc.vector.tensor_tensor if last else nc.gpsimd.tensor_tensor
            if last:
                nc.vector.tensor_tensor(a3(cmb)[:, gsl], a3(ss)[:, gsl],
                                        qb3[:, gsl, 0:A], OP.add)
            else:
                nc.gpsimd.tensor_tensor(a3(cmb)[:, gsl], a3(ss)[:, gsl],
                                        qb3[:, gsl, 0:A], OP.add)
            nc.vector.tensor_reduce(q0sel[:, gsl], a3(cmb)[:, gsl], AX.X,
                                    OP.max)
            nc.vector.tensor_reduce(maxqn[:, gsl], qb3[:, gsl, 32:32 + A],
                                    AX.X, OP.max)
            tt1(t1[:, gsl], maxqn[:, gsl], fac[:, gsl], OP.mult)
            tt1(t2[:, gsl], t1[:, gsl], rewb_s[:, gsl], OP.add)
            tt1(diff[:, gsl], q0sel[:, gsl], t2[:, gsl], OP.subtract)
            tt1(sq[:, gsl], diff[:, gsl], diff[:, gsl], OP.mult)

        # --- main software-pipelined loop ---
        do_dma(0, split=True)
        PASS_PER_LOAD = 2 * LOADCOLS // CH   # 8 ticks per load
        tails_done = 0
        for t in range(T + 12):
            nt = t + 4
            if nt < T and nt % PASS_PER_LOAD == 0:
                do_dma(nt // PASS_PER_LOAD)
            if t == 0:
                nc.gpsimd.dma_start(out=actb_s, in_=actb)
                nc.gpsimd.dma_start(out=rewb_s, in_=rewb)
                nc.gpsimd.dma_start(out=s1b_s, in_=s1b)
            if t >= 6 and t % 2 == 0 and (t - 6) // 2 < NQ:
                ep_front((t - 6) // 2)
            if t == 10:
                nc.vector.tensor_scalar(donem, s1b_s, DONE, None, OP.is_equal)
                nc.vector.tensor_scalar(fac, donem, -DISC, DISC, OP.mult,
                                        OP.add)
            if 0 <= t - 3 < T:
                st_relu2(t - 3)
            if 0 <= t - 1 < T:
                st_relu1(t - 1)
            if t < T:
                st_mm1(t)
            if 0 <= t - 2 < T:
                st_mm2(t - 2)
            tg = t - 10
            if tg >= 0 and tg % 4 == 0 and tg // 4 < NQG:
                st_tp(tg // 4)
            if 0 <= t - 5 < T and (t - 5) % 2 == 1:
                st_mm3((t - 5) // 2)
            tg = t - 9
            if tg >= 0 and tg % 4 == 0 and tg // 4 < NQG:
                st_stack(tg // 4)
            tg = t - 11
            if tg >= 0 and tg % 4 == 0 and tg // 4 < NQG:
                g = tg // 4
                st_qb(g)
                while (tails_done < NQ - 1
                       and (g + 1) * 8 >= (tails_done + 1) * HG):
                    ep_tail(tails_done)
                    tails_done += 1
                    if tails_done == NQ - 1:
                        nc.sync.dma_start(out=outp[:, 0:3 * HG],
                                          in_=sq[:, 0:3 * HG])
        while tails_done < NQ:
            ep_tail(tails_done)
            tails_done += 1
            if tails_done == NQ - 1:
                nc.sync.dma_start(out=outp[:, 0:3 * HG],
                                  in_=sq[:, 0:3 * HG])
        nc.sync.dma_start(out=outp[:, 3 * HG:GI], in_=sq[:, 3 * HG:GI])

    nc.compile()
    return nc


_CACHE = {}


def _get_program():
    if "nc" not in _CACHE:
        _CACHE["nc"] = _build_program()
    return _CACHE["nc"]


def _prep_in_maps(inputs):
    st0 = np.asarray(inputs["states0"], dtype=np.float32)
    st1 = np.asarray(inputs["states1"], dtype=np.float32)
    act = np.asarray(inputs["actions"], dtype=np.int32)
    rew = np.asarray(inputs["rewards"], dtype=np.float32)
    W1 = np.asarray(inputs["W1"], dtype=np.float32)
    W2 = np.asarray(inputs["W2"], dtype=np.float32)
    W3 = np.asarray(inputs["W3"], dtype=np.float32)

    s1_feat0 = st1[:, 0].copy()
    st1m = st1.copy()
    st1m[:, 0] = np.where(s1_feat0 == DONE, 0.0, s1_feat0)

    # mm1-DR layouts: x_dr[p, i, n] = x[n, i*64+p]; w1dr[p, m, i, mo] =
    # W1[i*64+p, m*128+mo]
    if L1_FP8:
        def xdr(st):
            xt = st.T.reshape(2, 64, B)  # [i, p, row]
            out = np.ascontiguousarray(xt.transpose(1, 0, 2))  # [64, 2, B]
            return np.clip(out, -240.0, 240.0).astype(NPF8)
        x0a = xdr(st0)
        x1a = xdr(st1m)
        x0dr = [np.ascontiguousarray(
                    x0a[:, :, c * BC:(c + 1) * BC]).reshape(64, 2 * BC)
                for c in range(NCORES)]
        x1dr = [np.ascontiguousarray(
                    x1a[:, :, c * BC:(c + 1) * BC]).reshape(64, 2 * BC)
                for c in range(NCORES)]
        w1p = np.empty((64, 2, 2, 128), np.float32)
        for m in range(2):
            for i in range(2):
                w1p[:, m, i, :] = W1[i * 64:i * 64 + 64,
                                     m * 128:(m + 1) * 128]
        w1b = w1p.reshape(64, 512).astype(NPF8)
    else:
        x0a = st0.T.astype(NPBF)
        x1a = st1m.T.astype(NPBF)
        x0dr = [np.ascontiguousarray(x0a[:, c * BC:(c + 1) * BC])
                for c in range(NCORES)]
        x1dr = [np.ascontiguousarray(x1a[:, c * BC:(c + 1) * BC])
                for c in range(NCORES)]
        w1b = W1.astype(NPBF)
    if L2_FP8:
        # [128, 2, 2*128]: w2d[:, m*256+(i*128+mo)] = W2[i*128+p, m*128+mo]
        w2p = np.empty((128, 2, 2, 128), np.float32)
        for m in range(2):
            for i in range(2):
                w2p[:, m, i, :] = W2[i * 128:(i + 1) * 128,
                                     m * 128:(m + 1) * 128]
        w2prep = w2p.reshape(128, 512).astype(NPF8)
    else:
        w2prep = W2.astype(NPBF)
    if L3_FP8:
        w3p = np.zeros((128, 2, 32), np.float32)
        for i in range(2):
            w3p[:, i, 0:A] = W3[i * 128:(i + 1) * 128, :]
        w3prep = w3p.reshape(128, 64).astype(NPF8)
    else:
        w3p = np.zeros((H, 32), np.float32)
        w3p[:, 0:A] = W3
        w3prep = w3p.astype(NPBF)

    iota = np.ascontiguousarray(
        np.broadcast_to(np.arange(A, dtype=np.float32) * 16.0, (128, A)))
    ident = np.eye(128, dtype=np.float32).astype(NPBF)

    # epilogue row permutation: row(r', gi) with gi=(g,j,p):
    #   g=gi//8, j=(gi%8)//2, p=gi%2 -> row = 512*(2g+p) + 128j + r'
    gi_idx = np.arange(GI)
    g_, j_, p_ = gi_idx // 8, (gi_idx % 8) // 2, gi_idx % 2
    base = 512 * (2 * g_ + p_) + 128 * j_          # [GI]
    rows = base[None, :] + np.arange(128)[:, None]  # [128, GI]

    in_maps = []
    for c in range(NCORES):
        r0, r1 = c * BC, (c + 1) * BC
        act_c = act[r0:r1]
        rew_c = rew[r0:r1]
        s1f_c = s1_feat0[r0:r1]
        in_maps.append({
            "x0t": x0dr[c], "x1t": x1dr[c],
            "actb": np.ascontiguousarray(
                act_c[rows].astype(np.float32).reshape(128, GI * A) * 512.0),
            "rewb": np.ascontiguousarray(rew_c[rows]),
            "s1b": np.ascontiguousarray(s1f_c[rows]),
            "w1d": w1b, "w2d": w2prep, "w3d": w3prep,
            "iotad": iota, "identd": ident,
        })
    return in_maps


def _run(inputs, trace=False):
    nc = _get_program()
    in_maps = _prep_in_maps(inputs)
    res = run_bass_kernel_spmd(nc, in_maps, core_ids=list(range(NCORES)),
                               trace=trace)
    total = 0.0
    for r in res.results:
        total += float(np.asarray(r["outp"], dtype=np.float64).sum())
    return np.array(np.float32(total)), res


def kernel(**inputs) -> np.ndarray:
    val, _ = _run(inputs, trace=False)
    return val
